# revision 1
# baseline (speedup 1.0000x reference)
"""Multi-head attention TRN2 kernel, 8-core (batch x head-block) sharded.

Problem (hardcoded): x[2,2048,1024] f32, Wq/Wk/Wv[1024,1024], Wo[1024,1024],
16 heads, dh=64. Reference computes softmax(Q K^T)/sqrt(1024) @ V @ Wo with the
division AFTER softmax (folded here into Wo as a host-side 1/32 scale).

Sharding: core c handles batch b=c//4 and head block hb=c%4 (4 heads = 256 dims:
Wq/Wk/Wv column slice, Wo row slice). Each core emits a partial Y[2048,1024];
host sums the 4 partials per batch.
"""

import numpy as np

import concourse.tile as tile
from concourse import bacc, mybir
from concourse.bass_utils import run_bass_kernel_spmd

N_CORES = 8
B = 2
S = 2048          # tokens per batch (= per core)
D = 1024          # model dim
DH = 64           # head dim
HPC = 4           # heads per core
DL = HPC * DH     # 256 local output dims per core
NG = DL // 128    # 2 partition groups of local dims
NK = D // 128     # 8 k-strips for QKV contraction
NT = S // 128     # 16 token strips
VW = 128          # V block: cols 0:64 = V dims, cols 64:128 = ones (denominators)

F32 = mybir.dt.float32
DT = mybir.dt.float32r   # PE fast fp32 mode
EXP = mybir.ActivationFunctionType.Exp
MULT = mybir.AluOpType.mult
ADD = mybir.AluOpType.add


def build_nc(repeat=1, phases=(0, 1, 2, 3)):
    nc = bacc.Bacc("TRN2", target_bir_lowering=False, debug=False)
    xT = nc.declare_dram_parameter("xT", [D, S], DT, isOutput=False)
    Wq = nc.declare_dram_parameter("Wq", [D, DL], DT, isOutput=False)
    Wk = nc.declare_dram_parameter("Wk", [D, DL], DT, isOutput=False)
    Wv = nc.declare_dram_parameter("Wv", [D, DL], DT, isOutput=False)
    Wo = nc.declare_dram_parameter("Wo", [DL, D], DT, isOutput=False)
    Ones = nc.declare_dram_parameter("onesv", [128, HPC * NT * VW], DT, isOutput=False)
    Yp = nc.declare_dram_parameter("Yp", [S, D], F32, isOutput=True)

    with tile.TileContext(nc) as tc:
        with tc.tile_pool(name="singles", bufs=1) as singles:
            wq_sb = singles.tile([128, NK * NG * 128], DT)
            wk_sb = singles.tile([128, NK * NG * 128], DT)
            wv_sb = singles.tile([128, NK * NG * 128], DT)
            wo_sb = singles.tile([128, NG * D], DT)
            qt_sb = singles.tile([128, NG * S], DT)
            # kt_z: per-head zero-padded keys. Block h = [128, S]; rows
            # (h%2)*64..+64 hold K dims, the other 64 rows are zero so QK
            # matmuls can contract the full 128 partitions (K=64 mms are 3.4x
            # slower than K=128 on this target).
            kt_z = singles.tile([128, HPC * S], DT)
            ot_sb = singles.tile([128, NG * S], DT)
            vaug_sb = singles.tile([128, HPC * NT * VW], DT)

            def body():
                # weight loads: col block (k*NG+g) of w_sb holds
                # W[k*128:(k+1)*128, g*128:(g+1)*128]
                if 0 in phases:
                    for w_dram, w_sb in ((Wq, wq_sb), (Wk, wk_sb), (Wv, wv_sb)):
                        for k in range(NK):
                            for g in range(NG):
                                cb = (k * NG + g) * 128
                                nc.sync.dma_start(
                                    out=w_sb[:, cb:cb + 128],
                                    in_=w_dram[k * 128:(k + 1) * 128,
                                               g * 128:(g + 1) * 128],
                                )
                    for g in range(NG):
                        nc.sync.dma_start(
                            out=wo_sb[:, g * D:(g + 1) * D],
                            in_=Wo[g * 128:(g + 1) * 128, :],
                        )
                    # memset can't target f32r; DMA ones from DRAM instead
                    nc.sync.dma_start(out=vaug_sb[:], in_=Ones[:, :])

                # ---- Phase 1: Q,K dim-major -> {q,k}t_sb; V token-major ----
                if 1 in phases:
                    phase1()
                if 2 in phases:
                    phase2()
                if 3 in phases:
                    phase3()

            def phase1():
                # zero the unused 64-row half of each kt_z head block
                # (memset can't write f32r; multiply a loaded tile by 0)
                for g in range(NG):
                    nc.vector.tensor_scalar(
                        kt_z[64:128, (2 * g) * S:(2 * g + 1) * S],
                        wv_sb[64:128, 0:S], 0.0, 0.0, MULT, ADD)
                    nc.vector.tensor_scalar(
                        kt_z[0:64, (2 * g + 1) * S:(2 * g + 2) * S],
                        wv_sb[0:64, 0:S], 0.0, 0.0, MULT, ADD)
                with tc.tile_pool(name="pp1", bufs=3, space="PSUM") as pp1, \
                     tc.tile_pool(name="ppv", bufs=3, space="PSUM") as ppv, \
                     tc.tile_pool(name="xkp", bufs=16) as xkp:
                    copy_engines = (nc.scalar, nc.vector)
                    ci = 0
                    for c in range(S // 512):
                        xks = []
                        for k in range(NK):
                            xk = xkp.tile([128, 512], DT, name="xk")
                            nc.sync.dma_start(
                                out=xk[:],
                                in_=xT[k * 128:(k + 1) * 128,
                                       c * 512:(c + 1) * 512],
                            )
                            xks.append(xk)
                        for w_sb, o_sb in ((wq_sb, qt_sb), (wk_sb, None)):
                            for g in range(NG):
                                ps = pp1.tile([128, 512], F32, name="ps_qk")
                                for k in range(NK):
                                    cb = (k * NG + g) * 128
                                    nc.tensor.matmul(
                                        ps[:],
                                        w_sb[:, cb:cb + 128],
                                        xks[k][:],
                                        start=(k == 0),
                                        stop=(k == NK - 1),
                                    )
                                cs = slice(c * 512, (c + 1) * 512)
                                if o_sb is not None:
                                    eng = copy_engines[ci % 2]
                                    ci += 1
                                    dst = o_sb[:, g * S + cs.start:
                                               g * S + cs.stop]
                                    if eng is nc.scalar:
                                        eng.copy(out=dst, in_=ps[:])
                                    else:
                                        eng.tensor_copy(out=dst, in_=ps[:])
                                else:
                                    nc.scalar.copy(
                                        out=kt_z[0:64, 2 * g * S + cs.start:
                                                 2 * g * S + cs.stop],
                                        in_=ps[0:64, :])
                                    nc.vector.tensor_copy(
                                        out=kt_z[64:128,
                                                 (2 * g + 1) * S + cs.start:
                                                 (2 * g + 1) * S + cs.stop],
                                        in_=ps[64:128, :])
                        for t in range(4):
                            j = c * 4 + t
                            pv = ppv.tile([128, DL], F32, name="pv")
                            for k in range(NK):
                                nc.tensor.matmul(
                                    pv[:],
                                    xks[k][:, t * 128:(t + 1) * 128],
                                    wv_sb[:, k * DL:(k + 1) * DL],
                                    start=(k == 0),
                                    stop=(k == NK - 1),
                                )
                            for h in range(HPC):
                                vb = (h * NT + j) * VW
                                nc.vector.tensor_copy(
                                    out=vaug_sb[:, vb:vb + DH],
                                    in_=pv[:, h * DH:(h + 1) * DH],
                                )

            def phase2():
                # attention, software-pipelined by one strip.
                # For each (head h, query-half sh): S^T strips [key128, query
                # 1024], exp on ACT, PV accumulation over 16 strips into
                # psum_O [66,1024] (row 64 = denominators via ones column).
                strips = [(h, sh, j) for h in range(HPC)
                          for sh in range(2) for j in range(NT)]
                pO_tiles = {}
                prev = None

                def emit_pv(expst, h, sh, j):
                    pO = pO_tiles[(h, sh)]
                    vb0 = h * NT * VW
                    for sc in range(2):
                        nc.tensor.matmul(
                            pO[:, sc * 512:(sc + 1) * 512],
                            vaug_sb[:, vb0 + j * VW: vb0 + (j + 1) * VW],
                            expst[:, sc * 512:(sc + 1) * 512],
                            start=(j == 0),
                            stop=(j == NT - 1),
                            skip_group_check=True,
                        )

                def emit_norm(h, sh):
                    # pO rows 64:128 all hold the softmax denominator (vaug
                    # cols 64:128 are ones), so no partition broadcast needed.
                    pO = pO_tiles.pop((h, sh))
                    g, r = h // 2, (h % 2) * DH
                    den = pO[DH:128, :]
                    rb = normp.tile([DH, 1024], F32, name="rb")
                    e = normp.tile([DH, 1024], F32, name="e")
                    nc.vector.reciprocal(rb[:], den)
                    # DVE reciprocal is ~2^-5 accurate; two Newton steps fix it
                    for _ in range(2):
                        nc.vector.tensor_tensor(out=e[:], in0=den, in1=rb[:],
                                                op=MULT)
                        nc.vector.tensor_scalar(e[:], e[:], -1.0, 2.0, MULT, ADD)
                        nc.vector.tensor_tensor(out=rb[:], in0=rb[:], in1=e[:],
                                                op=MULT)
                    nc.vector.tensor_tensor(
                        out=ot_sb[r:r + DH,
                                  g * S + sh * 1024: g * S + (sh + 1) * 1024],
                        in0=pO[0:DH, :],
                        in1=rb[:],
                        op=MULT,
                    )

                with tc.tile_pool(name="pS", bufs=2, space="PSUM") as pSp, \
                     tc.tile_pool(name="pO", bufs=2, space="PSUM") as pOp, \
                     tc.tile_pool(name="expp", bufs=3) as expp, \
                     tc.tile_pool(name="normp", bufs=2) as normp:
                    for h, sh, j in strips:
                        if j == 0:
                            pO_tiles[(h, sh)] = pOp.tile([VW, 1024], F32,
                                                         name="pO")
                        g = h // 2
                        pS = pSp.tile([128, 1024], F32, name="pS")
                        for sc in range(2):
                            nc.tensor.matmul(
                                pS[:, sc * 512:(sc + 1) * 512],
                                kt_z[:, h * S + j * 128: h * S + (j + 1) * 128],
                                qt_sb[:, g * S + sh * 1024 + sc * 512:
                                      g * S + sh * 1024 + (sc + 1) * 512],
                            )
                        expst = expp.tile([128, 1024], DT, name="expst")
                        nc.scalar.activation(expst[:], pS[:], EXP)
                        if prev is not None:
                            emit_pv(*prev)
                            _, ph, psh, pj = prev
                            if pj == NT - 1:
                                emit_norm(ph, psh)
                        prev = (expst, h, sh, j)
                    emit_pv(*prev)
                    emit_norm(prev[1], prev[2])

            def phase3():
                # output projection Y = OT^T @ Wo
                with tc.tile_pool(name="pY", bufs=4, space="PSUM") as pYp, \
                     tc.tile_pool(name="ysbp", bufs=4) as ysbp:
                    cnt = 0
                    for t in range(NT):
                        for e in range(2):
                            pY = pYp.tile([128, 512], F32, name="pY")
                            for g in range(NG):
                                nc.tensor.matmul(
                                    pY[:],
                                    ot_sb[:, g * S + t * 128:
                                          g * S + (t + 1) * 128],
                                    wo_sb[:, g * D + e * 512:
                                          g * D + (e + 1) * 512],
                                    start=(g == 0),
                                    stop=(g == NG - 1),
                                )
                            ysb = ysbp.tile([128, 512], F32, name="ysb")
                            if cnt % 2 == 0:
                                nc.scalar.copy(out=ysb[:], in_=pY[:])
                            else:
                                nc.vector.tensor_copy(out=ysb[:], in_=pY[:])
                            cnt += 1
                            nc.sync.dma_start(
                                out=Yp[t * 128:(t + 1) * 128,
                                       e * 512:(e + 1) * 512],
                                in_=ysb[:],
                            )

            for _ in range(repeat):
                body()
    nc.finalize()
    return nc


def make_in_maps(x, Wq, Wk, Wv, Wo):
    f = np.float32
    x = np.asarray(x, f)
    Wq, Wk, Wv, Wo = (np.asarray(a, f) for a in (Wq, Wk, Wv, Wo))
    in_maps = []
    xTs = [np.ascontiguousarray(x[b].T) for b in range(B)]
    onesv = np.ones((128, HPC * NT * VW), f)
    for c in range(N_CORES):
        b, hb = divmod(c, N_CORES // B)
        cols = slice(hb * DL, (hb + 1) * DL)
        in_maps.append({
            "xT": xTs[b],
            "Wq": np.ascontiguousarray(Wq[:, cols]),
            "Wk": np.ascontiguousarray(Wk[:, cols]),
            "Wv": np.ascontiguousarray(Wv[:, cols]),
            "Wo": np.ascontiguousarray(Wo[cols, :]) * f(1.0 / 32.0),
            "onesv": onesv,
        })
    return in_maps


def run(inputs, trace=False, repeat=1):
    nc = build_nc(repeat=repeat)
    in_maps = make_in_maps(**inputs)
    res = run_bass_kernel_spmd(nc, in_maps, list(range(N_CORES)), trace=trace)
    yps = [res.results[c]["Yp"] for c in range(N_CORES)]
    out = np.empty((B, S, D), np.float32)
    cpb = N_CORES // B
    for b in range(B):
        out[b] = sum(yps[b * cpb:(b + 1) * cpb])
    return out, res


def kernel(**inputs):
    out, _ = run(inputs, trace=False)
    return out



# revision 3
# speedup vs baseline: 1.1090x; 1.1090x over previous
"""Multi-head attention TRN2 kernel, 8-core (batch x head-block) sharded.

Problem (hardcoded): x[2,2048,1024] f32, Wq/Wk/Wv[1024,1024], Wo[1024,1024],
16 heads, dh=64. Reference computes softmax(Q K^T)/sqrt(1024) @ V @ Wo with the
division AFTER softmax (folded here into Wo as a host-side 1/32 scale).

Sharding: core c handles batch b=c//4 and head block hb=c%4 (4 heads = 256 dims:
Wq/Wk/Wv column slice, Wo row slice). Each core emits a partial Y[2048,1024]
in fp16; host sums the 4 partials per batch in fp32.

v2: bf16 datapath (within the 2e-2 error budget; measured ~9e-3), 64x128
row-tiled QK^T (two heads concurrently on PE tiles T0/T8 since dh=64),
single-op approx reciprocal for the softmax denominator, ones built on-chip
instead of a 4MB DRAM ones DMA, fp16 output partials (halves the output DMA).
"""

import numpy as np
import ml_dtypes

import concourse.tile as tile
from concourse import bacc, mybir
from concourse.bass_utils import run_bass_kernel_spmd

N_CORES = 8
B = 2
S = 2048          # tokens per batch (= per core)
D = 1024          # model dim
DH = 64           # head dim
HPC = 4           # heads per core
NP = HPC // 2     # head pairs per core (pair p = heads 2p, 2p+1)
DL = HPC * DH     # 256 local output dims per core
NK = D // 128     # 8 k-strips for QKV contraction
NT = S // 128     # 16 token strips
VW = 128          # V block: cols 0:64 = V dims, cols 64:128 = ones (denoms)

F32 = mybir.dt.float32
F16 = mybir.dt.float16
BF16 = mybir.dt.bfloat16
EXP = mybir.ActivationFunctionType.Exp
MULT = mybir.AluOpType.mult


def build_nc():
    nc = bacc.Bacc("TRN2", target_bir_lowering=False, debug=False)
    xT = nc.declare_dram_parameter("xT", [D, S], BF16, isOutput=False)
    Wq = nc.declare_dram_parameter("Wq", [D, DL], BF16, isOutput=False)
    Wk = nc.declare_dram_parameter("Wk", [D, DL], BF16, isOutput=False)
    Wv = nc.declare_dram_parameter("Wv", [D, DL], BF16, isOutput=False)
    Wo = nc.declare_dram_parameter("Wo", [DL, D], BF16, isOutput=False)
    Yp = nc.declare_dram_parameter("Yp", [S, D], F16, isOutput=True)

    with tile.TileContext(nc) as tc:
        with tc.tile_pool(name="singles", bufs=1) as singles:
            wq_sb = singles.tile([128, NK * NP * 128], BF16)
            wk_sb = singles.tile([128, NK * NP * 128], BF16)
            wv_sb = singles.tile([128, NK * DL], BF16)
            wo_sb = singles.tile([128, NP * D], BF16)
            # qt/kt: pair p block = cols [p*S, (p+1)*S); rows 0:64 = head 2p
            # dims, rows 64:128 = head 2p+1 dims (feeds PE tiles T0/T8).
            qt_sb = singles.tile([128, NP * S], BF16)
            kt_sb = singles.tile([128, NP * S], BF16)
            ot_sb = singles.tile([128, NP * S], BF16)
            vaug_sb = singles.tile([128, HPC * NT * VW], BF16)

            def body():
                # ones columns of vaug (denominator rows of pO); on-chip,
                # gpsimd is otherwise idle
                for blk in range(HPC * NT):
                    nc.gpsimd.memset(
                        vaug_sb[:, blk * VW + DH:(blk + 1) * VW], 1.0)

                # weight loads: col block (k*NP+g) of w_sb holds
                # W[k*128:(k+1)*128, g*128:(g+1)*128]
                for w_dram, w_sb in ((Wq, wq_sb), (Wk, wk_sb)):
                    for k in range(NK):
                        for g in range(NP):
                            cb = (k * NP + g) * 128
                            nc.sync.dma_start(
                                out=w_sb[:, cb:cb + 128],
                                in_=w_dram[k * 128:(k + 1) * 128,
                                           g * 128:(g + 1) * 128],
                            )
                for k in range(NK):
                    nc.sync.dma_start(
                        out=wv_sb[:, k * DL:(k + 1) * DL],
                        in_=Wv[k * 128:(k + 1) * 128, :],
                    )
                for g in range(NP):
                    nc.sync.dma_start(
                        out=wo_sb[:, g * D:(g + 1) * D],
                        in_=Wo[g * 128:(g + 1) * 128, :],
                    )
                phase1()
                phase2()
                phase3()

            def phase1():
                # Q,K dim-major -> {q,k}t_sb; V token-major -> vaug
                with tc.tile_pool(name="pp1", bufs=3, space="PSUM") as pp1, \
                     tc.tile_pool(name="ppv", bufs=3, space="PSUM") as ppv, \
                     tc.tile_pool(name="xkp", bufs=16) as xkp:
                    copy_engines = (nc.scalar, nc.vector)
                    ci = 0
                    for c in range(S // 512):
                        xks = []
                        for k in range(NK):
                            xk = xkp.tile([128, 512], BF16, name="xk")
                            nc.sync.dma_start(
                                out=xk[:],
                                in_=xT[k * 128:(k + 1) * 128,
                                       c * 512:(c + 1) * 512],
                            )
                            xks.append(xk)
                        for w_sb, o_sb in ((wq_sb, qt_sb), (wk_sb, kt_sb)):
                            for g in range(NP):
                                ps = pp1.tile([128, 512], F32, name="ps_qk")
                                for k in range(NK):
                                    cb = (k * NP + g) * 128
                                    nc.tensor.matmul(
                                        ps[:],
                                        w_sb[:, cb:cb + 128],
                                        xks[k][:],
                                        start=(k == 0),
                                        stop=(k == NK - 1),
                                    )
                                cs = slice(c * 512, (c + 1) * 512)
                                eng = copy_engines[ci % 2]
                                ci += 1
                                dst = o_sb[:, g * S + cs.start:
                                           g * S + cs.stop]
                                if eng is nc.scalar:
                                    eng.copy(out=dst, in_=ps[:])
                                else:
                                    eng.tensor_copy(out=dst, in_=ps[:])
                        for t in range(4):
                            j = c * 4 + t
                            pv = ppv.tile([128, DL], F32, name="pv")
                            for k in range(NK):
                                nc.tensor.matmul(
                                    pv[:],
                                    xks[k][:, t * 128:(t + 1) * 128],
                                    wv_sb[:, k * DL:(k + 1) * DL],
                                    start=(k == 0),
                                    stop=(k == NK - 1),
                                )
                            for h in range(HPC):
                                vb = (h * NT + j) * VW
                                eng = copy_engines[ci % 2]
                                ci += 1
                                dst = vaug_sb[:, vb:vb + DH]
                                src = pv[:, h * DH:(h + 1) * DH]
                                if eng is nc.scalar:
                                    eng.copy(out=dst, in_=src)
                                else:
                                    eng.tensor_copy(out=dst, in_=src)

            def phase2():
                # attention, software-pipelined by one head-strip.
                # Per (pair p, query-half sh): strips j over keys; the two
                # heads of a pair run concurrently on PE row tiles T0
                # (partitions 0:64) and T8 (64:128) since dh=64. exp on ACT
                # (the phase-2 bottleneck), PV accumulation over 16 strips
                # into pO [128,1024] (rows 64:128 = softmax denominators via
                # vaug ones cols).
                strips = [(p, sh, j, half)
                          for sh in range(2) for p in range(NP)
                          for j in range(NT) for half in range(2)]
                pO_tiles = {}
                prev = None

                def emit_pv(expst, p, sh, j, half):
                    pO = pO_tiles[(p, sh, half)]
                    h = 2 * p + half
                    vb = (h * NT + j) * VW
                    for sc in range(2):
                        nc.tensor.matmul(
                            pO[:, sc * 512:(sc + 1) * 512],
                            vaug_sb[:, vb:vb + VW],
                            expst[:, sc * 512:(sc + 1) * 512],
                            start=(j == 0),
                            stop=(j == NT - 1),
                            skip_group_check=True,
                        )

                def emit_norm(p, sh, half):
                    # pO rows 64:128 all hold the softmax denominator (vaug
                    # cols 64:128 are ones): single approx reciprocal
                    # (~18 bits) then scale the V rows.
                    pO = pO_tiles.pop((p, sh, half))
                    r = half * DH
                    # custom-DVE ops drop the input partition offset, so run
                    # the reciprocal over all 128 partitions (rows 0:64 are
                    # V sums — garbage reciprocals there, discarded).
                    rb = normp.tile([128, 1024], F32, name="rb")
                    nc.vector.reciprocal_approx_fast(rb[:], pO[:])
                    nc.vector.tensor_tensor(
                        out=ot_sb[r:r + DH,
                                  p * S + sh * 1024:p * S + (sh + 1) * 1024],
                        in0=pO[0:DH, :],
                        in1=rb[DH:128, :],
                        op=MULT,
                    )

                with tc.tile_pool(name="pS", bufs=2, space="PSUM") as pSp, \
                     tc.tile_pool(name="pO", bufs=2, space="PSUM") as pOp, \
                     tc.tile_pool(name="expp", bufs=4) as expp, \
                     tc.tile_pool(name="normp", bufs=2) as normp:
                    for p, sh, j, half in strips:
                        if j == 0 and half == 0:
                            pO_tiles[(p, sh, 0)] = pOp.tile([VW, 1024], F32,
                                                            name="pO")
                            pO_tiles[(p, sh, 1)] = pOp.tile([VW, 1024], F32,
                                                            name="pO")
                        r0 = half * DH
                        pS = pSp.tile([128, 1024], F32, name="pS")
                        for sc in range(2):
                            nc.tensor.matmul(
                                pS[:, sc * 512:(sc + 1) * 512],
                                kt_sb[r0:r0 + DH,
                                      p * S + j * 128:p * S + (j + 1) * 128],
                                qt_sb[r0:r0 + DH,
                                      p * S + sh * 1024 + sc * 512:
                                      p * S + sh * 1024 + (sc + 1) * 512],
                            )
                        expst = expp.tile([128, 1024], BF16, name="expst")
                        nc.scalar.activation(expst[:], pS[:], EXP)
                        if prev is not None:
                            emit_pv(*prev)
                            _, pp_, psh, pj, phalf = prev
                            if pj == NT - 1:
                                emit_norm(pp_, psh, phalf)
                        prev = (expst, p, sh, j, half)
                    emit_pv(*prev)
                    emit_norm(prev[1], prev[2], prev[4])

            def phase3():
                # output projection Y = OT^T @ Wo, fp16 partials out
                with tc.tile_pool(name="pY", bufs=4, space="PSUM") as pYp, \
                     tc.tile_pool(name="ysbp", bufs=4) as ysbp:
                    cnt = 0
                    for t in range(NT):
                        for e in range(2):
                            pY = pYp.tile([128, 512], F32, name="pY")
                            for g in range(NP):
                                nc.tensor.matmul(
                                    pY[:],
                                    ot_sb[:, g * S + t * 128:
                                          g * S + (t + 1) * 128],
                                    wo_sb[:, g * D + e * 512:
                                          g * D + (e + 1) * 512],
                                    start=(g == 0),
                                    stop=(g == NP - 1),
                                )
                            ysb = ysbp.tile([128, 512], F16, name="ysb")
                            if cnt % 2 == 0:
                                nc.scalar.copy(out=ysb[:], in_=pY[:])
                            else:
                                nc.vector.tensor_copy(out=ysb[:], in_=pY[:])
                            cnt += 1
                            nc.sync.dma_start(
                                out=Yp[t * 128:(t + 1) * 128,
                                       e * 512:(e + 1) * 512],
                                in_=ysb[:],
                            )

            body()
    nc.finalize()
    return nc


def make_in_maps(x, Wq, Wk, Wv, Wo):
    bf = ml_dtypes.bfloat16
    f = np.float32
    x = np.asarray(x, f)
    Wq, Wk, Wv, Wo = (np.asarray(a, f) for a in (Wq, Wk, Wv, Wo))
    in_maps = []
    xTs = [np.ascontiguousarray(x[b].T).astype(bf) for b in range(B)]
    for c in range(N_CORES):
        b, hb = divmod(c, N_CORES // B)
        cols = slice(hb * DL, (hb + 1) * DL)
        in_maps.append({
            "xT": xTs[b],
            "Wq": np.ascontiguousarray(Wq[:, cols]).astype(bf),
            "Wk": np.ascontiguousarray(Wk[:, cols]).astype(bf),
            "Wv": np.ascontiguousarray(Wv[:, cols]).astype(bf),
            "Wo": (np.ascontiguousarray(Wo[cols, :]) * f(1.0 / 32.0)).astype(bf),
        })
    return in_maps


def run(inputs, trace=False):
    nc = build_nc()
    in_maps = make_in_maps(**inputs)
    res = run_bass_kernel_spmd(nc, in_maps, list(range(N_CORES)), trace=trace)
    yps = [res.results[c]["Yp"] for c in range(N_CORES)]
    out = np.empty((B, S, D), np.float32)
    cpb = N_CORES // B
    for b in range(B):
        out[b] = np.sum([yps[b * cpb + i].astype(np.float32)
                         for i in range(cpb)], axis=0)
    return out, res


def kernel(**inputs):
    out, _ = run(inputs, trace=False)
    return out
